# revision 26
# baseline (speedup 1.0000x reference)
"""Distributed attention forward kernel for one TRN2 chip (8 NeuronCores).

Problem: B=4, L=2048, D_IN=1024, 16 heads x 64 dim.
  qk = (x @ Wqk + bqk) / 32            -> q,k per head
  v  = (x @ Wv + bv) / 32
  out = softmax(q k^T / 64) v          -> [B, L, 1024]

Sharding: core c handles batch c//2 and heads 8*(c%2) .. +8
(data parallel over batch x tensor parallel over heads). No collectives;
the host scatters inputs and gathers the per-core [2048, 512] outputs.

Measured per-instruction rates on this part (from ntff issue spacing):
  matmul K=128 N=512 bf16: ~216 ns    K=64 N=512 (start|stop): ~226 ns
  matmul fp8 DoubleRow [128,2,*] N=512 (K=256): ~226 ns  (2x flops)
  activation exp [128,1024] PSUM->SBUF: ~1146 ns
The exp stream (256 tiles) is a ~293 us Activation-engine wall; the PE
budget (S 116 + AV 123 + QKproj-fp8 29 + V 28 + transposes 12 us) sits
just under it, so the kernel is paced by exp except at the edges.

Per-core dataflow (one NeuronCore, Tile-scheduled):
  1. x^T arrives twice from the host: bf16 (for the V projection / exact
     path) and fp8e4 in chunk-pair layout [128,2,L] (for the qk
     projection). Four DMA queues split the input load so the first S
     matmul can issue ~9 us in.
  2. qk^T = Wqk^T x^T at NATURAL weight scale (no 1/32 folding: fp8e4
     normals start at 2^-6) via fp8 DoubleRow matmuls (K=256 per
     instruction). All scale factors fold into the exp:
     exp(S_raw / 65536). Host column permutation stacks heads in pairs
     as in the bf16 version.
  3. v = x @ Wv' bf16 in natural [pos, cols] layout with a fused
     ones-column per head ([v_h | 1]) so the AV matmul also produces the
     softmax denominator.
  4. Per head, per 1024-wide q block, per 128-wide k chunk:
       S^T = matmul(lhsT=k^T chunk, rhs=q^T)    [128 k, 1024 q]
       E   = exp(S^T / 65536) on ScalarE -> bf16
       psum_O += matmul(lhsT=[v|1] chunk, rhs=E)
     AV lags S/exp by two chunks. Remaining qk projection pieces (fp8 DR,
     4 matmuls each) drip in between chunks; a few discarded pieces pad
     the tail so the HAM activity monitor never sees a sparse PE array.
  5. psum_O [65, q] -> SBUF (bf16) -> PE-transpose (bf16, 1 cyc/row) to
     [q, 65]; row 64 is the denominator: reciprocal + per-partition
     scalar multiply, one batched output DMA per (head, q block).
"""

import sys

if "/opt/trn_rl_repo" not in sys.path:
    sys.path.insert(0, "/opt/trn_rl_repo")

from collections import deque
from contextlib import ExitStack

import ml_dtypes
import numpy as np

import concourse.bass as bass
import concourse.mybir as mybir
from concourse import bacc
from concourse.tile import TileContext

# Problem constants (hardcoded; kernel.py must be self-contained).
B = 4
L = 2048
D_IN = 1024
HEADS = 16
DIM = 64
N_CORES = 8

H_LOC = 8          # heads per core
PAIRS = 4          # head pairs per core
QK_COLS = 1024     # 8 heads * 128 (q+k) columns per core
V_COLS = 512       # 8 heads * 64
VE_COLS = H_LOC * (DIM + 1)  # 520, v plus ones column per head

F32 = mybir.dt.float32
BF16 = mybir.dt.bfloat16
F8 = mybir.dt.float8e4
DR = mybir.MatmulPerfMode.DoubleRow

# qk projection runs at natural weight scale; the two 1/sqrt(d_in)
# factors and the 1/64 logit scale all fold into the exp input scale.
EXP_SCALE = 1.0 / (64.0 * 1024.0)


def build_nc():
    nc = bacc.Bacc()

    xt_e = nc.declare_dram_parameter("xt", [D_IN, L], BF16, isOutput=False)
    xt8_e = nc.declare_dram_parameter("xt8", [D_IN, L], F8, isOutput=False)
    wqk_e = nc.declare_dram_parameter("wqk", [D_IN, QK_COLS], F8, isOutput=False)
    bqk_e = nc.declare_dram_parameter("bqk2", [128, 8], F32, isOutput=False)
    wv_e = nc.declare_dram_parameter("wv", [D_IN, V_COLS], BF16, isOutput=False)
    bve_e = nc.declare_dram_parameter("bve", [128, VE_COLS], F32, isOutput=False)
    id_e = nc.declare_dram_parameter("ident", [128, 128], BF16, isOutput=False)
    out_e = nc.declare_dram_parameter("out", [L, V_COLS], F32, isOutput=True)

    with TileContext(nc) as tc, ExitStack() as ctx:
        singles = ctx.enter_context(tc.tile_pool(name="singles", bufs=1))
        p_xt = ctx.enter_context(tc.tile_pool(name="xt", bufs=8))
        p_xt8 = ctx.enter_context(tc.tile_pool(name="xt8", bufs=4))
        p_wqk = ctx.enter_context(tc.tile_pool(name="wqkp", bufs=8))
        p_wv = ctx.enter_context(tc.tile_pool(name="wvp", bufs=8))
        p_qkt = ctx.enter_context(tc.tile_pool(name="qkt", bufs=8))
        p_vext = ctx.enter_context(tc.tile_pool(name="vext", bufs=16))
        # E tiles live until their (deferred, ~1.5 blocks later) AV pops
        # consume them; the pool depth caps the backlog depth via WAR
        # dependencies if draining falls behind.
        p_e = ctx.enter_context(tc.tile_pool(name="epool", bufs=36))
        p_otsb = ctx.enter_context(tc.tile_pool(name="otsb", bufs=2))
        p_outt = ctx.enter_context(tc.tile_pool(name="outt", bufs=2))
        p_rec = ctx.enter_context(tc.tile_pool(name="rec", bufs=4))
        pp_a = ctx.enter_context(tc.tile_pool(name="ppa", bufs=3, space="PSUM"))
        pp_ot = ctx.enter_context(tc.tile_pool(name="ppot", bufs=1, space="PSUM"))

        # Four DMA queues; per-queue issue is in emission order, so the
        # critical chain (wqk0 -> xt8 -> first S) leads its queues.
        wqk_t = [None] * 8

        def load_wqk(c, eng):
            w = p_wqk.tile([128, 8, 128], F8, name=f"wqk{c}", tag="wqk")
            eng.dma_start(
                out=w,
                in_=wqk_e.ap()
                .rearrange("(kc p) q -> p kc q", p=128)[
                    :, :, c * 128 : (c + 1) * 128
                ],
            )
            wqk_t[c] = w

        def load_xt8(t, eng):
            xt8[t] = p_xt8.tile([128, 2, L], F8, name=f"xt8_{t}", tag="xt8")
            eng.dma_start(
                out=xt8[t],
                in_=xt8_e[256 * t : 256 * t + 256, :].rearrange(
                    "(i p) l -> p i l", i=2
                ),
            )

        def load_xt(dc, eng):
            xt[dc] = p_xt.tile([128, L], BF16, name=f"xt{dc}", tag="xt")
            eng.dma_start(out=xt[dc], in_=xt_e[dc * 128 : (dc + 1) * 128, :])

        # DMA plan. The HBM round-robins all outstanding descriptors, so
        # the first-S critical set (wqk0/1, bias, the four xt8 tiles,
        # 2.26MB) must be alone in flight: it leads all three queues,
        # everything else sits behind it in the same queues' FIFOs (sync,
        # which is otherwise idle) or behind an explicit gate copy that
        # waits for xt8_0 (gpsimd).
        xt8 = [None] * 4
        xt = [None] * 8
        load_wqk(0, nc.sync)
        load_wqk(1, nc.sync)
        bqk_sb = singles.tile([128, 8], F32)
        nc.sync.dma_start(out=bqk_sb, in_=bqk_e[:, :])
        load_xt8(0, nc.sync)
        load_xt8(1, nc.scalar)
        load_xt8(3, nc.scalar)
        bve_sb = singles.tile([128, VE_COLS], F32)
        nc.scalar.dma_start(out=bve_sb, in_=bve_e[:, :])
        ident = singles.tile([128, 128], BF16)
        nc.scalar.dma_start(out=ident, in_=id_e[:, :])
        load_xt8(2, nc.gpsimd)

        # sync tail: remaining wqk chunks (drip, needed from block ~3),
        # then half the bf16 x^T chunks.
        for c in range(2, 8):
            load_wqk(c, nc.sync)
        for dc in (7, 6, 5, 4):
            load_xt(dc, nc.sync)

        # gpsimd tail, gated behind xt8_0 completion so the bulk doesn't
        # steal HBM from the critical set.
        gate_sb = singles.tile([128, 4], F8)
        nc.gpsimd.tensor_copy(gate_sb, xt8[0][:, 0, 0:4])
        wv_t = []
        for kc in range(8):
            w = p_wv.tile([128, V_COLS], BF16, name=f"wv{kc}", tag="wv")
            nc.gpsimd.dma_start(out=w, in_=wv_e[kc * 128 : (kc + 1) * 128, :])
            wv_t.append(w)
        for dc in (0, 1, 2, 3):
            load_xt(dc, nc.gpsimd)

        # qk^T output tiles: chunk 2p = q^T of pair p, chunk 2p+1 = k^T.
        qk_t = [
            p_qkt.tile([128, L], BF16, name=f"qkt{c}", tag="qkt") for c in range(8)
        ]
        # v (+ ones col) tiles, one per 128-position chunk, bf16.
        ve_t = [
            p_vext.tile([128, VE_COLS], BF16, name=f"ve{i}", tag="ve")
            for i in range(16)
        ]

        def qk_piece_mm(psq, c, kk, pc2):
            # One fp8 DoubleRow matmul: contracts d_in chunks 2kk,2kk+1.
            nc.tensor.matmul(
                psq,
                wqk_t[c][:, 2 * kk : 2 * kk + 2, :],
                xt8[kk][:, :, pc2 * 512 : pc2 * 512 + 512],
                start=(kk == 0),
                stop=(kk == 3),
                perf_mode=DR,
            )

        def project_qk_piece(c, pc2):
            psq = pp_a.tile([128, 512], F32, tag="ps", bufs=2)
            for kk in range(4):
                qk_piece_mm(psq, c, kk, pc2)
            nc.vector.tensor_scalar_add(
                qk_t[c][:, pc2 * 512 : pc2 * 512 + 512],
                psq,
                bqk_sb[:, c : c + 1],
            )

        # v-chunk projection, split into two backlog pops (the 1.7us of
        # matmuls would overshoot a slot's PE budget and stall the exp
        # stream). ptr bank only: psq belongs to the multi-slot drip
        # pieces; vp halves pop on consecutive slots, so the bufs=1 WAR
        # on the prior chunk's bias-copy is already resolved.
        vp_state = {"psv": None}

        def project_v_half(pc, half):
            if half == 0:
                vp_state["psv"] = pp_a.tile([128, V_COLS], F32, name="psv", tag="ptr", bufs=1)
            psv = vp_state["psv"]
            for kc in (0, 1, 2, 3) if half == 0 else (4, 5, 6, 7):
                nc.tensor.matmul(
                    psv,
                    xt[kc][:, pc * 128 : pc * 128 + 128],
                    wv_t[kc],
                    start=(kc == 0),
                    stop=(kc == 7),
                )
            if half == 0:
                return
            ve = ve_t[pc]
            nc.vector.tensor_tensor(
                ve.rearrange("p (h d) -> p h d", h=H_LOC)[:, :, 0:DIM],
                psv.rearrange("p (h d) -> p h d", h=H_LOC),
                bve_sb.rearrange("p (h d) -> p h d", h=H_LOC)[:, :, 0:DIM],
                mybir.AluOpType.add,
            )
            nc.vector.tensor_copy(
                ve.rearrange("p (h d) -> p h d", h=H_LOC)[:, :, DIM : DIM + 1],
                bve_sb.rearrange("p (h d) -> p h d", h=H_LOC)[:, :, DIM : DIM + 1],
            )
            vp_state["psv"] = None

        # Block-0 JIT pieces, emitted in HALVES (2 DR matmuls per kcp
        # slot; a whole piece overshoots the slot's PE budget and stalls
        # the exp stream). Nothing else touches the psq bank in block 0.
        jit_state = {"psq": None}

        def project_qk_half(c, pc2, half):
            if half == 0:
                jit_state["psq"] = pp_a.tile([128, 512], F32, name="jitq", tag="psq", bufs=1)
            for kk in (0, 1) if half == 0 else (2, 3):
                qk_piece_mm(jit_state["psq"], c, kk, pc2)
            if half == 1:
                nc.vector.tensor_scalar_add(
                    qk_t[c][:, pc2 * 512 : pc2 * 512 + 512],
                    jit_state["psq"],
                    bqk_sb[:, c : c + 1],
                )
                jit_state["psq"] = None

        # Drip queue: remaining qk projection pieces (4 DR matmuls each)
        # in DEADLINE order (head pair p's kt chunk 0 and first q columns
        # are read at block 4p's first slots), then discarded pieces pad
        # the tail so the HAM monitor never sees a sparse PE array.
        drip_queue = []
        for c, pc2 in [
            (3, 0), (2, 0), (2, 1), (3, 1), (3, 2), (3, 3), (2, 2), (2, 3),
            (5, 0), (4, 0), (4, 1), (5, 1), (5, 2), (5, 3), (4, 2), (4, 3),
            (7, 0), (6, 0), (6, 1), (7, 1), (7, 2), (7, 3), (6, 2), (6, 3),
        ]:
            drip_queue.append((c, pc2, False))
        drip = {"pos": 0, "kk": 0, "psq": None}

        def drip_mm():
            if drip["pos"] >= len(drip_queue):
                return
            c, pc2, dummy = drip_queue[drip["pos"]]
            if drip["kk"] == 0:
                drip["psq"] = pp_a.tile([128, 512], F32, name="dripq", tag="psq", bufs=1)
            kk = drip["kk"]
            qk_piece_mm(drip["psq"], c, kk, pc2)
            drip["kk"] += 1
            if drip["kk"] == 4:
                if not dummy:
                    nc.vector.tensor_scalar_add(
                        qk_t[c][:, pc2 * 512 : pc2 * 512 + 512],
                        drip["psq"],
                        bqk_sb[:, c : c + 1],
                    )
                drip["psq"] = None
                drip["kk"] = 0
                drip["pos"] += 1

        def _av(ps_ot, ve, et, lh, kc):
            for i in range(2):
                nc.tensor.matmul(
                    ps_ot[:, i * 512 : i * 512 + 512],
                    ve[:, lh * 65 : lh * 65 + 65],
                    et[:, i * 512 : i * 512 + 512],
                    start=(kc == 0),
                    stop=(kc == 15),
                )

        # ---- backlog: deferred AV / v-projection / finale work ----
        # The exp stream on ScalarE is the pacing engine; the PE queue is
        # in-order, so anything that might stall (v-proj waiting on x^T
        # DMAs, AV waiting on a just-issued exp, finale chains) must sit
        # BEHIND the current block's S matmuls in emission order. Each
        # block emits only its S/exp pairs on schedule; its AV
        # accumulations and finale are queued here and popped in measured
        # doses (about one unit ~500ns of PE per k-chunk slot) during
        # LATER slots, where all their dependencies are a full block old.
        backlog = deque()

        def pop_units(budget):
            while backlog and budget > 0:
                u, fn = backlog.popleft()
                fn()
                budget -= u

        def make_av(ots, kc, ve, et, lh):
            def fn():
                if ots["t"] is None:
                    ots["t"] = pp_ot.tile([65, 1024], F32, name="ps_ot", tag="ot")
                _av(ots["t"], ve, et, lh, kc)
            return fn

        def make_finale(ots, lh, qh):
            # Three pops: PSUM->SBUF copy, then two groups of four
            # transpose+reciprocal+scale columns (the second also emits
            # the batched output DMA).
            fin = {}

            def fn_copy():
                ot_sb = p_otsb.tile([65, 1024], BF16, tag="ot_sb")
                nc.vector.tensor_copy(ot_sb, ots["t"])
                fin["ot_sb"] = ot_sb
                fin["ott"] = p_outt.tile([128, 8, DIM], F32, name="ott", tag="ott")

            def group(qcs, dma):
                def fn():
                    ot_sb, ott = fin["ot_sb"], fin["ott"]
                    for qc in qcs:
                        ptr = pp_a.tile([128, 65], BF16, tag="ptr", bufs=1)
                        nc.tensor.transpose(
                            ptr, ot_sb[:, qc * 128 : qc * 128 + 128],
                            ident[0:65, 0:65],
                        )
                        rec = p_rec.tile([128, 1], F32, tag="rec")
                        nc.vector.reciprocal(rec, ptr[:, 64:65])
                        nc.vector.tensor_scalar_mul(ott[:, qc, :], ptr[:, 0:DIM], rec)
                    if dma:
                        nc.sync.dma_start(
                            out=out_e.ap().rearrange("(qq p) n -> p qq n", p=128)[
                                :, 8 * qh : 8 * qh + 8, lh * DIM : (lh + 1) * DIM
                            ],
                            in_=fin["ott"],
                        )
                return fn

            return [(1, fn_copy), (2, group(range(4), False)),
                    (2, group(range(4, 8), True))]

        # Warm-up: keep the PE array DENSELY active from the moment the
        # engines come up (~6us, reading a memset scratch tile -- no DMA
        # dependency) until the xt8 tiles land and the projection starts.
        # The p-state ramp needs ~3us of sustained activity and the HAM
        # monitor halves the clock if the array goes sparse.
        scratch = singles.tile([128, 512], BF16)
        nc.gpsimd.memset(scratch, 1.0)
        for i in range(26):
            pw = pp_a.tile([128, 512], F32, name="warm", tag="ps", bufs=2)
            nc.tensor.matmul(
                pw,
                scratch[:, 0:128],
                scratch,
                start=True,
                stop=True,
            )

        # Minimum upfront projection: q cols 0-1023 of pair 0, k positions
        # 0-511, and block 1's first q-column piece; the rest is emitted
        # just-in-time in halves inside block 0 (jit_sched) or dripped.
        project_qk_piece(0, 0)
        project_qk_piece(0, 1)
        project_qk_piece(1, 0)
        project_qk_piece(0, 2)

        # Block-0 JIT piece halves, keyed by kcp slot. kt chunk 4k..4k+3
        # is read by S at kc=4k, so piece (1,k) completes at kcp 2k-1;
        # (0,3) is block 1's q columns.
        jit_sched = {
            0: ((1, 1), 0), 1: ((1, 1), 1),
            2: ((1, 2), 0), 3: ((1, 2), 1),
            4: ((1, 3), 0), 5: ((1, 3), 1),
            6: ((0, 3), 0), 7: ((0, 3), 1),
        }

        blocks = [
            (p, hh, qh)
            for p in range(PAIRS)
            for hh, qh in [(0, 0), (0, 1), (1, 0), (1, 1)]
        ]
        for bi, (p, hh, qh) in enumerate(blocks):
            lh = 2 * p + hh
            qt, kt = qk_t[2 * p], qk_t[2 * p + 1]
            prow = slice(hh * 64, hh * 64 + 64)
            q0 = qh * 1024
            ots = {"t": None}
            e_list = []

            def s_exp(kc):
                ps = pp_a.tile([128, 1024], F32, tag="ps", bufs=2)
                for i in range(2):
                    nc.tensor.matmul(
                        ps[:, i * 512 : i * 512 + 512],
                        kt[prow, kc * 128 : kc * 128 + 128],
                        qt[prow, q0 + i * 512 : q0 + i * 512 + 512],
                        start=True,
                        stop=True,
                    )
                et = p_e.tile([128, 1024], BF16, tag="E")
                nc.scalar.activation(
                    et, ps, mybir.ActivationFunctionType.Exp, scale=EXP_SCALE
                )
                e_list.append(et)

            for kcp in range(8):
                s_exp(2 * kcp)
                s_exp(2 * kcp + 1)
                if bi == 0 and kcp in jit_sched:
                    (c, pc2), half = jit_sched[kcp]
                    project_qk_half(c, pc2, half)
                pop_units(6 if bi >= 14 else (4 if bi < 3 else 3))
                if bi >= 3:
                    drip_mm()
                    drip_mm()

            # Queue this block's deferred work: block 0 also owns the 16
            # v-chunk projections (interleaved so ve_t[kc] is written
            # before its first AV reader).
            for kc in range(16):
                if bi == 0:
                    for half in (0, 1):
                        backlog.append(
                            (2, (lambda c, h: (lambda: project_v_half(c, h)))(kc, half))
                        )
                backlog.append((1, make_av(ots, kc, ve_t[kc], e_list[kc], lh)))
            backlog.extend(make_finale(ots, lh, qh))

        while backlog:
            pop_units(8)

    nc.compile()
    return nc


def host_prep(x, Wqk, bqk, Wv, bv, core):
    """Per-core input shard with host-folded scales and layouts."""
    b = core // 2
    base = (core % 2) * H_LOC
    s = np.float32(1.0 / 32.0)  # 1 / d_in**0.5

    cols = []
    for p in range(PAIRS):
        g0 = base + 2 * p
        g1 = g0 + 1
        cols.extend(range(g0 * 128, g0 * 128 + 64))
        cols.extend(range(g1 * 128, g1 * 128 + 64))
        cols.extend(range(g0 * 128 + 64, g0 * 128 + 128))
        cols.extend(range(g1 * 128 + 64, g1 * 128 + 128))
    cols = np.asarray(cols)

    # qk projection at natural scale (scales fold into the exp input).
    wqk_d = np.ascontiguousarray(Wqk[:, cols].astype(ml_dtypes.float8_e4m3))
    bqk_d = np.ascontiguousarray(bqk[cols].reshape(8, 128).T, dtype=np.float32)
    wv_d = np.ascontiguousarray(
        (Wv[:, base * DIM : (base + H_LOC) * DIM] * s).astype(ml_dtypes.bfloat16)
    )
    bve = np.zeros((H_LOC, DIM + 1), np.float32)
    bve[:, :DIM] = (bv[base * DIM : (base + H_LOC) * DIM] * s).reshape(H_LOC, DIM)
    bve[:, DIM] = 1.0
    bve_d = np.ascontiguousarray(
        np.broadcast_to(bve.reshape(1, VE_COLS), (128, VE_COLS)), dtype=np.float32
    )
    xt = np.ascontiguousarray(x[b].T.astype(ml_dtypes.bfloat16))
    return {
        "xt": xt,
        "xt8": np.ascontiguousarray(x[b].T.astype(ml_dtypes.float8_e4m3)),
        "wqk": wqk_d,
        "bqk2": bqk_d,
        "wv": wv_d,
        "bve": bve_d,
        "ident": np.eye(128, dtype=ml_dtypes.bfloat16),
    }


_NC_CACHE = None


def _get_nc():
    global _NC_CACHE
    if _NC_CACHE is None:
        _NC_CACHE = build_nc()
    return _NC_CACHE


def run(inputs, **spmd_kwargs):
    """Run on the 8 NeuronCores; returns (full_output, BassKernelResults)."""
    from concourse.bass_utils import run_bass_kernel_spmd

    x = np.asarray(inputs["x"], dtype=np.float32)
    wqk = np.asarray(inputs["Wqk"], dtype=np.float32)
    bqk = np.asarray(inputs["bqk"], dtype=np.float32)
    wv = np.asarray(inputs["Wv"], dtype=np.float32)
    bv = np.asarray(inputs["bv"], dtype=np.float32)

    in_maps = [host_prep(x, wqk, bqk, wv, bv, c) for c in range(N_CORES)]
    nc = _get_nc()
    res = run_bass_kernel_spmd(nc, in_maps, core_ids=list(range(N_CORES)), **spmd_kwargs)

    out = np.empty((B, L, HEADS * DIM), np.float32)
    for c in range(N_CORES):
        b = c // 2
        base = (c % 2) * H_LOC
        out[b][:, base * DIM : (base + H_LOC) * DIM] = res.results[c]["out"]
    return out, res


def kernel(**inputs):
    out, _ = run(inputs)
    return out


# revision 30
# speedup vs baseline: 1.0185x; 1.0185x over previous
"""Distributed attention forward kernel for one TRN2 chip (8 NeuronCores).

Problem: B=4, L=2048, D_IN=1024, 16 heads x 64 dim.
  qk = (x @ Wqk + bqk) / 32            -> q,k per head
  v  = (x @ Wv + bv) / 32
  out = softmax(q k^T / 64) v          -> [B, L, 1024]

Sharding: core c handles batch c//2 and heads 8*(c%2) .. +8
(data parallel over batch x tensor parallel over heads). No collectives;
the host scatters inputs and gathers the per-core [2048, 512] outputs.

Measured per-instruction rates on this part (from ntff issue spacing):
  matmul K=128 N=512 bf16: ~216 ns    K=64 N=512 (start|stop): ~226 ns
  matmul fp8 DoubleRow [128,2,*] N=512 (K=256): ~226 ns  (2x flops)
  activation exp [128,1024] PSUM->SBUF: ~1146 ns
The exp stream (256 tiles) is a ~293 us Activation-engine wall; the PE
budget (S 116 + AV 123 + QKproj-fp8 29 + V 28 + transposes 12 us) sits
just under it, so the kernel is paced by exp except at the edges.

Per-core dataflow (one NeuronCore, Tile-scheduled):
  1. x^T arrives twice from the host: bf16 (for the V projection / exact
     path) and fp8e4 in chunk-pair layout [128,2,L] (for the qk
     projection). Four DMA queues split the input load so the first S
     matmul can issue ~9 us in.
  2. qk^T = Wqk^T x^T at NATURAL weight scale (no 1/32 folding: fp8e4
     normals start at 2^-6) via fp8 DoubleRow matmuls (K=256 per
     instruction). All scale factors fold into the exp:
     exp(S_raw / 65536). Host column permutation stacks heads in pairs
     as in the bf16 version.
  3. v = x @ Wv' bf16 in natural [pos, cols] layout with a fused
     ones-column per head ([v_h | 1]) so the AV matmul also produces the
     softmax denominator.
  4. Per head, per 1024-wide q block, per 128-wide k chunk:
       S^T = matmul(lhsT=k^T chunk, rhs=q^T)    [128 k, 1024 q]
       E   = exp(S^T / 65536) on ScalarE -> bf16
       psum_O += matmul(lhsT=[v|1] chunk, rhs=E)
     AV lags S/exp by two chunks. Remaining qk projection pieces (fp8 DR,
     4 matmuls each) drip in between chunks; a few discarded pieces pad
     the tail so the HAM activity monitor never sees a sparse PE array.
  5. psum_O [65, q] -> SBUF (bf16) -> PE-transpose (bf16, 1 cyc/row) to
     [q, 65]; row 64 is the denominator: reciprocal + per-partition
     scalar multiply, one batched output DMA per (head, q block).
"""

import sys

if "/opt/trn_rl_repo" not in sys.path:
    sys.path.insert(0, "/opt/trn_rl_repo")

from collections import deque
from contextlib import ExitStack

import ml_dtypes
import numpy as np

import concourse.bass as bass
import concourse.mybir as mybir
from concourse import bacc
from concourse.tile import TileContext

# Problem constants (hardcoded; kernel.py must be self-contained).
B = 4
L = 2048
D_IN = 1024
HEADS = 16
DIM = 64
N_CORES = 8

H_LOC = 8          # heads per core
PAIRS = 4          # head pairs per core
QK_COLS = 1024     # 8 heads * 128 (q+k) columns per core
V_COLS = 512       # 8 heads * 64
VE_COLS = H_LOC * (DIM + 1)  # 520, v plus ones column per head

F32 = mybir.dt.float32
BF16 = mybir.dt.bfloat16
F8 = mybir.dt.float8e4
DR = mybir.MatmulPerfMode.DoubleRow

# qk projection runs at natural weight scale; the two 1/sqrt(d_in)
# factors and the 1/64 logit scale all fold into the exp input scale.
EXP_SCALE = 1.0 / (64.0 * 1024.0)


def build_nc():
    nc = bacc.Bacc()

    xt_e = nc.declare_dram_parameter("xt", [D_IN, L], BF16, isOutput=False)
    xt8_e = nc.declare_dram_parameter("xt8", [D_IN, L], F8, isOutput=False)
    wqk_e = nc.declare_dram_parameter("wqk", [D_IN, QK_COLS], F8, isOutput=False)
    bqk_e = nc.declare_dram_parameter("bqk2", [128, 8], F32, isOutput=False)
    wv_e = nc.declare_dram_parameter("wv", [D_IN, V_COLS], BF16, isOutput=False)
    bve_e = nc.declare_dram_parameter("bve", [128, VE_COLS], F32, isOutput=False)
    id_e = nc.declare_dram_parameter("ident", [128, 128], BF16, isOutput=False)
    out_e = nc.declare_dram_parameter("out", [L, V_COLS], F32, isOutput=True)

    with TileContext(nc) as tc, ExitStack() as ctx:
        singles = ctx.enter_context(tc.tile_pool(name="singles", bufs=1))
        p_xt = ctx.enter_context(tc.tile_pool(name="xt", bufs=8))
        p_xt8 = ctx.enter_context(tc.tile_pool(name="xt8", bufs=4))
        p_wqk = ctx.enter_context(tc.tile_pool(name="wqkp", bufs=8))
        p_wv = ctx.enter_context(tc.tile_pool(name="wvp", bufs=8))
        p_qkt = ctx.enter_context(tc.tile_pool(name="qkt", bufs=8))
        p_vext = ctx.enter_context(tc.tile_pool(name="vext", bufs=16))
        # E tiles live until their (deferred, ~1.5 blocks later) AV pops
        # consume them; the pool depth caps the backlog depth via WAR
        # dependencies if draining falls behind.
        p_e = ctx.enter_context(tc.tile_pool(name="epool", bufs=36))
        p_otsb = ctx.enter_context(tc.tile_pool(name="otsb", bufs=2))
        p_outt = ctx.enter_context(tc.tile_pool(name="outt", bufs=2))
        p_rec = ctx.enter_context(tc.tile_pool(name="rec", bufs=4))
        pp_a = ctx.enter_context(tc.tile_pool(name="ppa", bufs=3, space="PSUM"))
        pp_ot = ctx.enter_context(tc.tile_pool(name="ppot", bufs=1, space="PSUM"))

        # Four DMA queues; per-queue issue is in emission order, so the
        # critical chain (wqk0 -> xt8 -> first S) leads its queues.
        wqk_t = [None] * 8

        def load_wqk(c, eng):
            w = p_wqk.tile([128, 8, 128], F8, name=f"wqk{c}", tag="wqk")
            eng.dma_start(
                out=w,
                in_=wqk_e.ap()
                .rearrange("(kc p) q -> p kc q", p=128)[
                    :, :, c * 128 : (c + 1) * 128
                ],
            )
            wqk_t[c] = w

        def load_xt8(t, eng):
            xt8[t] = p_xt8.tile([128, 2, L], F8, name=f"xt8_{t}", tag="xt8")
            eng.dma_start(
                out=xt8[t],
                in_=xt8_e[256 * t : 256 * t + 256, :].rearrange(
                    "(i p) l -> p i l", i=2
                ),
            )

        def load_xt(dc, eng):
            xt[dc] = p_xt.tile([128, L], BF16, name=f"xt{dc}", tag="xt")
            eng.dma_start(out=xt[dc], in_=xt_e[dc * 128 : (dc + 1) * 128, :])

        # DMA plan. The HBM round-robins all outstanding descriptors, so
        # the first-S critical set (wqk0/1, bias, the four xt8 tiles,
        # 2.26MB) must be alone in flight: it leads all three queues,
        # everything else sits behind it in the same queues' FIFOs (sync,
        # which is otherwise idle) or behind an explicit gate copy that
        # waits for xt8_0 (gpsimd).
        xt8 = [None] * 4
        xt = [None] * 8
        load_wqk(0, nc.sync)
        load_wqk(1, nc.sync)
        bqk_sb = singles.tile([128, 8], F32)
        nc.sync.dma_start(out=bqk_sb, in_=bqk_e[:, :])
        load_xt8(0, nc.sync)
        load_xt8(1, nc.scalar)
        load_xt8(3, nc.scalar)
        bve_sb = singles.tile([128, VE_COLS], F32)
        nc.scalar.dma_start(out=bve_sb, in_=bve_e[:, :])
        ident = singles.tile([128, 128], BF16)
        nc.scalar.dma_start(out=ident, in_=id_e[:, :])
        load_xt8(2, nc.gpsimd)

        # sync tail: remaining wqk chunks (drip, needed from block ~3),
        # then half the bf16 x^T chunks.
        for c in range(2, 8):
            load_wqk(c, nc.sync)
        for dc in (7, 6, 5, 4):
            load_xt(dc, nc.sync)

        # gpsimd tail, gated behind xt8_0 completion so the bulk doesn't
        # steal HBM from the critical set.
        gate_sb = singles.tile([128, 4], F8)
        nc.gpsimd.tensor_copy(gate_sb, xt8[0][:, 0, 0:4])
        wv_t = []
        for kc in range(8):
            w = p_wv.tile([128, V_COLS], BF16, name=f"wv{kc}", tag="wv")
            nc.gpsimd.dma_start(out=w, in_=wv_e[kc * 128 : (kc + 1) * 128, :])
            wv_t.append(w)
        for dc in (0, 1, 2, 3):
            load_xt(dc, nc.gpsimd)

        # qk^T output tiles: chunk 2p = q^T of pair p, chunk 2p+1 = k^T.
        qk_t = [
            p_qkt.tile([128, L], BF16, name=f"qkt{c}", tag="qkt") for c in range(8)
        ]
        # v (+ ones col) tiles, one per 128-position chunk, bf16.
        ve_t = [
            p_vext.tile([128, VE_COLS], BF16, name=f"ve{i}", tag="ve")
            for i in range(16)
        ]

        def qk_piece_mm(psq, c, kk, pc2):
            # One fp8 DoubleRow matmul: contracts d_in chunks 2kk,2kk+1.
            nc.tensor.matmul(
                psq,
                wqk_t[c][:, 2 * kk : 2 * kk + 2, :],
                xt8[kk][:, :, pc2 * 512 : pc2 * 512 + 512],
                start=(kk == 0),
                stop=(kk == 3),
                perf_mode=DR,
            )

        def project_qk_piece(c, pc2):
            psq = pp_a.tile([128, 512], F32, tag="ps", bufs=2)
            for kk in range(4):
                qk_piece_mm(psq, c, kk, pc2)
            nc.vector.tensor_scalar_add(
                qk_t[c][:, pc2 * 512 : pc2 * 512 + 512],
                psq,
                bqk_sb[:, c : c + 1],
            )

        # v-chunk projection, split into two backlog pops (the 1.7us of
        # matmuls would overshoot a slot's PE budget and stall the exp
        # stream). ptr bank only: psq belongs to the multi-slot drip
        # pieces; vp halves pop on consecutive slots, so the bufs=1 WAR
        # on the prior chunk's bias-copy is already resolved.
        vp_state = {"psv": None}

        def project_v_half(pc, half):
            if half == 0:
                vp_state["psv"] = pp_a.tile([128, V_COLS], F32, name="psv", tag="ptr", bufs=1)
            psv = vp_state["psv"]
            for kc in (0, 1, 2, 3) if half == 0 else (4, 5, 6, 7):
                nc.tensor.matmul(
                    psv,
                    xt[kc][:, pc * 128 : pc * 128 + 128],
                    wv_t[kc],
                    start=(kc == 0),
                    stop=(kc == 7),
                )
            if half == 0:
                return
            ve = ve_t[pc]
            nc.vector.tensor_tensor(
                ve.rearrange("p (h d) -> p h d", h=H_LOC)[:, :, 0:DIM],
                psv.rearrange("p (h d) -> p h d", h=H_LOC),
                bve_sb.rearrange("p (h d) -> p h d", h=H_LOC)[:, :, 0:DIM],
                mybir.AluOpType.add,
            )
            nc.vector.tensor_copy(
                ve.rearrange("p (h d) -> p h d", h=H_LOC)[:, :, DIM : DIM + 1],
                bve_sb.rearrange("p (h d) -> p h d", h=H_LOC)[:, :, DIM : DIM + 1],
            )
            vp_state["psv"] = None

        # Block-0 JIT pieces, emitted in HALVES (2 DR matmuls per kcp
        # slot; a whole piece overshoots the slot's PE budget and stalls
        # the exp stream). Nothing else touches the psq bank in block 0.
        jit_state = {"psq": None}

        def project_qk_half(c, pc2, half):
            if half == 0:
                jit_state["psq"] = pp_a.tile([128, 512], F32, name="jitq", tag="psq", bufs=1)
            for kk in (0, 1) if half == 0 else (2, 3):
                qk_piece_mm(jit_state["psq"], c, kk, pc2)
            if half == 1:
                nc.vector.tensor_scalar_add(
                    qk_t[c][:, pc2 * 512 : pc2 * 512 + 512],
                    jit_state["psq"],
                    bqk_sb[:, c : c + 1],
                )
                jit_state["psq"] = None

        # Drip queue: remaining qk projection pieces (4 DR matmuls each)
        # in DEADLINE order (head pair p's kt chunk 0 and first q columns
        # are read at block 4p's first slots), then discarded pieces pad
        # the tail so the HAM monitor never sees a sparse PE array.
        drip_queue = []
        for c, pc2 in [
            (3, 0), (2, 0), (2, 1), (3, 1), (3, 2), (3, 3), (2, 2), (2, 3),
            (5, 0), (4, 0), (4, 1), (5, 1), (5, 2), (5, 3), (4, 2), (4, 3),
            (7, 0), (6, 0), (6, 1), (7, 1), (7, 2), (7, 3), (6, 2), (6, 3),
        ]:
            drip_queue.append((c, pc2, False))
        drip = {"pos": 0, "kk": 0, "psq": None}

        def drip_mm():
            if drip["pos"] >= len(drip_queue):
                return
            c, pc2, dummy = drip_queue[drip["pos"]]
            if drip["kk"] == 0:
                drip["psq"] = pp_a.tile([128, 512], F32, name="dripq", tag="psq", bufs=1)
            kk = drip["kk"]
            qk_piece_mm(drip["psq"], c, kk, pc2)
            drip["kk"] += 1
            if drip["kk"] == 4:
                if not dummy:
                    nc.vector.tensor_scalar_add(
                        qk_t[c][:, pc2 * 512 : pc2 * 512 + 512],
                        drip["psq"],
                        bqk_sb[:, c : c + 1],
                    )
                drip["psq"] = None
                drip["kk"] = 0
                drip["pos"] += 1

        def _av(ps_ot, ve, et, lh, kc):
            for i in range(2):
                nc.tensor.matmul(
                    ps_ot[:, i * 512 : i * 512 + 512],
                    ve[:, lh * 65 : lh * 65 + 65],
                    et[:, i * 512 : i * 512 + 512],
                    start=(kc == 0),
                    stop=(kc == 15),
                )

        # ---- backlog: deferred AV / v-projection / finale work ----
        # The exp stream on ScalarE is the pacing engine; the PE queue is
        # in-order, so anything that might stall (v-proj waiting on x^T
        # DMAs, AV waiting on a just-issued exp, finale chains) must sit
        # BEHIND the current block's S matmuls in emission order. Each
        # block emits only its S/exp pairs on schedule; its AV
        # accumulations and finale are queued here and popped in measured
        # doses (about one unit ~500ns of PE per k-chunk slot) during
        # LATER slots, where all their dependencies are a full block old.
        backlog = deque()

        def pop_units(budget):
            while backlog and budget > 0:
                u, fn = backlog.popleft()
                fn()
                budget -= u

        def make_av(ots, kc, ve, et, lh):
            def fn():
                if ots["t"] is None:
                    ots["t"] = pp_ot.tile([65, 1024], F32, name="ps_ot", tag="ot")
                _av(ots["t"], ve, et, lh, kc)
            return fn

        def make_finale(ots, lh, qh):
            # Three pops: PSUM->SBUF copy, then two groups of four
            # transpose+reciprocal+scale columns (the second also emits
            # the batched output DMA).
            fin = {}

            def fn_copy():
                ot_sb = p_otsb.tile([65, 1024], BF16, tag="ot_sb")
                nc.vector.tensor_copy(ot_sb, ots["t"])
                fin["ot_sb"] = ot_sb
                fin["ott"] = p_outt.tile([128, 8, DIM], F32, name="ott", tag="ott")

            def group(qcs, dma):
                def fn():
                    ot_sb, ott = fin["ot_sb"], fin["ott"]
                    for qc in qcs:
                        ptr = pp_a.tile([128, 65], BF16, tag="ptr", bufs=1)
                        nc.tensor.transpose(
                            ptr, ot_sb[:, qc * 128 : qc * 128 + 128],
                            ident[0:65, 0:65],
                        )
                        rec = p_rec.tile([128, 1], F32, tag="rec")
                        nc.vector.reciprocal(rec, ptr[:, 64:65])
                        nc.vector.tensor_scalar_mul(ott[:, qc, :], ptr[:, 0:DIM], rec)
                    if dma:
                        nc.sync.dma_start(
                            out=out_e.ap().rearrange("(qq p) n -> p qq n", p=128)[
                                :, 8 * qh : 8 * qh + 8, lh * DIM : (lh + 1) * DIM
                            ],
                            in_=fin["ott"],
                        )
                return fn

            return [(1, fn_copy), (2, group(range(4), False)),
                    (2, group(range(4, 8), True))]

        # Warm-up: keep the PE array DENSELY active from the moment the
        # engines come up (~6us, reading a memset scratch tile -- no DMA
        # dependency) until the xt8 tiles land and the projection starts.
        # The p-state ramp needs ~3us of sustained activity and the HAM
        # monitor halves the clock if the array goes sparse.
        scratch = singles.tile([128, 512], BF16)
        nc.vector.memset(scratch, 1.0)
        for i in range(26):
            pw = pp_a.tile([128, 512], F32, name="warm", tag="ps", bufs=2)
            nc.tensor.matmul(
                pw,
                scratch[:, 0:128],
                scratch,
                start=True,
                stop=True,
            )

        # Minimum upfront projection: q cols 0-1023 of pair 0, k positions
        # 0-511, and block 1's first q-column piece; the rest is emitted
        # just-in-time in halves inside block 0 (jit_sched) or dripped.
        project_qk_piece(0, 0)
        project_qk_piece(0, 1)
        project_qk_piece(1, 0)
        project_qk_piece(0, 2)

        # Block-0 JIT piece halves, keyed by kcp slot. kt chunk 4k..4k+3
        # is read by S at kc=4k, so piece (1,k) completes at kcp 2k-1;
        # (0,3) is block 1's q columns.
        jit_sched = {
            0: ((1, 1), 0), 1: ((1, 1), 1),
            2: ((1, 2), 0), 3: ((1, 2), 1),
            4: ((1, 3), 0), 5: ((1, 3), 1),
            6: ((0, 3), 0), 7: ((0, 3), 1),
        }

        blocks = [
            (p, hh, qh)
            for p in range(PAIRS)
            for hh, qh in [(0, 0), (0, 1), (1, 0), (1, 1)]
        ]
        for bi, (p, hh, qh) in enumerate(blocks):
            lh = 2 * p + hh
            qt, kt = qk_t[2 * p], qk_t[2 * p + 1]
            prow = slice(hh * 64, hh * 64 + 64)
            q0 = qh * 1024
            ots = {"t": None}
            e_list = []

            def s_exp(kc):
                ps = pp_a.tile([128, 1024], F32, tag="ps", bufs=2)
                for i in range(2):
                    nc.tensor.matmul(
                        ps[:, i * 512 : i * 512 + 512],
                        kt[prow, kc * 128 : kc * 128 + 128],
                        qt[prow, q0 + i * 512 : q0 + i * 512 + 512],
                        start=True,
                        stop=True,
                    )
                et = p_e.tile([128, 1024], BF16, tag="E")
                nc.scalar.activation(
                    et, ps, mybir.ActivationFunctionType.Exp, scale=EXP_SCALE
                )
                e_list.append(et)

            av_fns = None
            if bi == 15:
                # Lazy: e_list fills as the kcp loop emits s_exp.
                def _mk(kc):
                    def fn():
                        if ots["t"] is None:
                            ots["t"] = pp_ot.tile(
                                [65, 1024], F32, name="ps_ot", tag="ot"
                            )
                        _av(ots["t"], ve_t[kc], e_list[kc], lh, kc)
                    return fn
                av_fns = [_mk(kc) for kc in range(16)]

            for kcp in range(8):
                s_exp(2 * kcp)
                s_exp(2 * kcp + 1)
                if bi == 0 and kcp in jit_sched:
                    (c, pc2), half = jit_sched[kcp]
                    project_qk_half(c, pc2, half)
                pop_units(8 if bi >= 14 else (4 if bi < 3 else 3))
                if bi == 15 and kcp >= 4:
                    # Inline lag-8 AVs once block 14's finale has popped,
                    # so the end-of-kernel drain is short.
                    av_fns[2 * (kcp - 4)]()
                    av_fns[2 * (kcp - 4) + 1]()
                if 3 <= bi < 15:
                    drip_mm()
                    drip_mm()

            # Queue this block's deferred work. Block 0 also owns the 16
            # v-chunk projections (interleaved so ve_t[kc] is written
            # before its first AV reader); its second half is pushed after
            # block 1's S-era so blocks 1 and 2 share the load evenly.
            if bi == 15:
                for kc in range(8, 16):
                    av_fns[kc]()
                for _, fn in make_finale(ots, lh, qh):
                    fn()
            else:
                def push_b0(kcs):
                    for kc in kcs:
                        for half in (0, 1):
                            backlog.append(
                                (2, (lambda c, h: (lambda: project_v_half(c, h)))(kc, half))
                            )
                        backlog.append(
                            (1, make_av(ots, kc, ve_t[kc], e_list[kc], lh))
                        )

                if bi == 0:
                    push_b0(range(8))
                    b0_tail = (ots, e_list)
                else:
                    if bi == 1:
                        o0, el0 = b0_tail
                        for kc in range(8, 16):
                            for half in (0, 1):
                                backlog.append(
                                    (2, (lambda c, h: (lambda: project_v_half(c, h)))(kc, half))
                                )
                            backlog.append(
                                (1, make_av(o0, kc, ve_t[kc], el0[kc], 0))
                            )
                        backlog.extend(make_finale(o0, 0, 0))

                    for kc in range(16):
                        backlog.append((1, make_av(ots, kc, ve_t[kc], e_list[kc], lh)))
                    backlog.extend(make_finale(ots, lh, qh))
            e_list_ref = e_list

        while backlog:
            pop_units(8)

    nc.compile()
    return nc


def host_prep(x, Wqk, bqk, Wv, bv, core):
    """Per-core input shard with host-folded scales and layouts."""
    b = core // 2
    base = (core % 2) * H_LOC
    s = np.float32(1.0 / 32.0)  # 1 / d_in**0.5

    cols = []
    for p in range(PAIRS):
        g0 = base + 2 * p
        g1 = g0 + 1
        cols.extend(range(g0 * 128, g0 * 128 + 64))
        cols.extend(range(g1 * 128, g1 * 128 + 64))
        cols.extend(range(g0 * 128 + 64, g0 * 128 + 128))
        cols.extend(range(g1 * 128 + 64, g1 * 128 + 128))
    cols = np.asarray(cols)

    # qk projection at natural scale (scales fold into the exp input).
    wqk_d = np.ascontiguousarray(Wqk[:, cols].astype(ml_dtypes.float8_e4m3))
    bqk_d = np.ascontiguousarray(bqk[cols].reshape(8, 128).T, dtype=np.float32)
    wv_d = np.ascontiguousarray(
        (Wv[:, base * DIM : (base + H_LOC) * DIM] * s).astype(ml_dtypes.bfloat16)
    )
    bve = np.zeros((H_LOC, DIM + 1), np.float32)
    bve[:, :DIM] = (bv[base * DIM : (base + H_LOC) * DIM] * s).reshape(H_LOC, DIM)
    bve[:, DIM] = 1.0
    bve_d = np.ascontiguousarray(
        np.broadcast_to(bve.reshape(1, VE_COLS), (128, VE_COLS)), dtype=np.float32
    )
    xt = np.ascontiguousarray(x[b].T.astype(ml_dtypes.bfloat16))
    return {
        "xt": xt,
        "xt8": np.ascontiguousarray(x[b].T.astype(ml_dtypes.float8_e4m3)),
        "wqk": wqk_d,
        "bqk2": bqk_d,
        "wv": wv_d,
        "bve": bve_d,
        "ident": np.eye(128, dtype=ml_dtypes.bfloat16),
    }


_NC_CACHE = None


def _get_nc():
    global _NC_CACHE
    if _NC_CACHE is None:
        _NC_CACHE = build_nc()
    return _NC_CACHE


def run(inputs, **spmd_kwargs):
    """Run on the 8 NeuronCores; returns (full_output, BassKernelResults)."""
    from concourse.bass_utils import run_bass_kernel_spmd

    x = np.asarray(inputs["x"], dtype=np.float32)
    wqk = np.asarray(inputs["Wqk"], dtype=np.float32)
    bqk = np.asarray(inputs["bqk"], dtype=np.float32)
    wv = np.asarray(inputs["Wv"], dtype=np.float32)
    bv = np.asarray(inputs["bv"], dtype=np.float32)

    in_maps = [host_prep(x, wqk, bqk, wv, bv, c) for c in range(N_CORES)]
    nc = _get_nc()
    res = run_bass_kernel_spmd(nc, in_maps, core_ids=list(range(N_CORES)), **spmd_kwargs)

    out = np.empty((B, L, HEADS * DIM), np.float32)
    for c in range(N_CORES):
        b = c // 2
        base = (c % 2) * H_LOC
        out[b][:, base * DIM : (base + H_LOC) * DIM] = res.results[c]["out"]
    return out, res


def kernel(**inputs):
    out, _ = run(inputs)
    return out
